# revision 13
# baseline (speedup 1.0000x reference)
"""Trainium2 Bass kernel for MF embedding-lookup + dot-product scoring.

out[u, i] = dot(user_hiddens[user_ids[u]], item_hiddens[item_ids[i]])

Sharding: 2D over 8 cores - 4 user groups (1024 users) x 2 item groups
(2048 items); tables replicated to every core's HBM. Per core:
  - users: 8 indirect-DMA gathers (128 f32 rows each) -> PE transpose ->
    bf16 cast -> uhi [64, 1024]
  - items: host converts the item table to bf16 with columns duplicated
    to 128 (256B rows, dma_gather's granularity) and range-buckets this
    core's item ids into 4 static 25600-row table slices; 4 bulk
    dma_gather calls (single_packet=False - large single packets crash
    the SWDGE) land 128-row bf16 tiles partition-major, one descriptor
    per row; int16 indices stay in range. ~1.2us/call vs ~1.1us per 128
    rows for indirect DMA.
  - per 128-item tile: bf16 PE transpose -> v^T, then 2 matmuls (K=64,
    N=512): lhsT = v^T, rhs = uhi halves -> f32 PSUM
  - PSUM -> SBUF fp16 casts alternate vector/scalar; per-bucket batched
    DMAs write only the real (non-pad) rows -> out [2048, 1024] fp16
Host un-permutes the bucketed item order, transposes, upcasts to f32,
and assembles the full [4096, 4096].
"""

import numpy as np
import ml_dtypes

import concourse.bacc as bacc
import concourse.bass as bass
import concourse.mybir as mybir
import concourse.tile as tile
from concourse.bass_utils import run_bass_kernel_spmd
from concourse.masks import make_identity

NUM_USERS = 1_000_000
NUM_ITEMS = 100_000
D = 64
E = 128             # bf16 item row width (256B, dma_gather granularity)
BU = 4096
BI = 4096
N_CORES = 8
RU = 4              # user groups
RI = 2              # item groups
UC = BU // RU       # users per core = 1024
IC = BI // RI       # items per core = 2048
P = 128
UT = UC // P        # user gather calls = 8
NBLK = 512          # matmul moving free dim (one PSUM bank of f32)
NH = UC // NBLK     # user halves per item tile = 2
NBUCKET = 4
BWIDTH = 25_600     # static item-table range per bucket (< 32768)

_cache = {}


def _ceil128(n):
    return (n + P - 1) // P * P


def _build(mks, nks):
    """mks: per-bucket padded index counts (x128); nks: real counts."""
    nc = bacc.Bacc()
    ut_dram = nc.dram_tensor(
        "user_table", [NUM_USERS, D], mybir.dt.float32, kind="ExternalInput"
    )
    it_dram = nc.dram_tensor(
        "item_dup", [NUM_ITEMS, E], mybir.dt.bfloat16, kind="ExternalInput"
    )
    uid_dram = nc.dram_tensor("uids", [P, UT], mybir.dt.int32, kind="ExternalInput")
    icols = sum(mks) // 16
    iidx_dram = nc.dram_tensor(
        "iidx", [P, icols], mybir.dt.int16, kind="ExternalInput"
    )
    out_rows = sum(nks)
    out_dram = nc.dram_tensor(
        "out", [out_rows, UC], mybir.dt.float16, kind="ExternalOutput"
    )

    f32 = mybir.dt.float32
    bf16 = mybir.dt.bfloat16
    fp16 = mybir.dt.float16

    with tile.TileContext(nc) as tc:
        with (
            tc.tile_pool(name="const", bufs=1) as constp,
            tc.tile_pool(name="idx", bufs=1) as idxp,
            tc.tile_pool(name="gath", bufs=1) as gathp,
            tc.tile_pool(name="ops", bufs=1) as opsp,
            tc.tile_pool(name="vt", bufs=4) as vtp,
            tc.tile_pool(name="tp", bufs=2, space="PSUM") as tpp,
            tc.tile_pool(name="mm", bufs=3, space="PSUM") as mmp,
            tc.tile_pool(name="outp", bufs=2) as outp,
        ):
            ident = constp.tile([P, P], f32)
            make_identity(nc, ident[:])
            ident_bf = constp.tile([P, P], bf16)
            make_identity(nc, ident_bf[:])

            uids = idxp.tile([P, UT], mybir.dt.int32)
            iidx = idxp.tile([P, icols], mybir.dt.int16)
            nc.sync.dma_start(out=uids[:], in_=uid_dram[:])
            nc.sync.dma_start(out=iidx[:], in_=iidx_dram[:])

            gu = gathp.tile([P, UT * D], f32)
            ntiles = sum(mks) // P
            vmov = gathp.tile([P, ntiles * E], bf16)

            def user_gather(t):
                nc.gpsimd.indirect_dma_start(
                    out=gu[:, t * D : (t + 1) * D],
                    out_offset=None,
                    in_=ut_dram[:],
                    in_offset=bass.IndirectOffsetOnAxis(
                        ap=uids[:, t : t + 1], axis=0
                    ),
                )

            def item_gather(k):
                toff = sum(mks[:k]) // P
                icoff = sum(mks[:k]) // 16
                nc.gpsimd.dma_gather(
                    out_ap=vmov[
                        :, toff * E : (toff + mks[k] // P) * E
                    ].rearrange("p (o n) -> p o n", n=E),
                    in_ap=it_dram[k * BWIDTH : min((k + 1) * BWIDTH, NUM_ITEMS), :],
                    idxs_ap=iidx[:, icoff : icoff + mks[k] // 16],
                    num_idxs=mks[k],
                    num_idxs_reg=mks[k],
                    elem_size=E,
                    transpose=False,
                    single_packet=False,
                )

            # gpsimd issue order: first half of users, first item bucket,
            # rest of users, remaining buckets
            for t in range(4):
                user_gather(t)
            item_gather(0)
            for t in range(4, UT):
                user_gather(t)
            for k in range(1, NBUCKET):
                item_gather(k)

            # --- user prologue: transpose + bf16 cast -> uhi [64, 1024] ---
            uhi = opsp.tile([D, UC], bf16)
            for t in range(UT):
                ps = tpp.tile([D, P], f32)
                nc.tensor.transpose(ps[:], gu[:, t * D : (t + 1) * D], ident[:])
                nc.vector.tensor_copy(
                    out=uhi[:, t * P : (t + 1) * P], in_=ps[:]
                )

            # --- item stream: bf16 transpose -> matmuls ---
            cp = 0  # copy-engine rotation counter
            tglob = 0
            for k in range(NBUCKET):
                tk = mks[k] // P          # tiles in this bucket
                fk = nks[k] // P          # full tiles
                rk = nks[k] % P           # real rows in partial tile
                rowoff = sum(nks[:k])     # output row offset
                ob = outp.tile([P, tk * UC], fp16)
                for j in range(tk):
                    t = tglob + j
                    ps = tpp.tile([D, P], bf16)
                    nc.tensor.transpose(
                        ps[:], vmov[:, t * E : t * E + D], ident_bf[:]
                    )
                    vst = vtp.tile([D, P], bf16)
                    nc.scalar.copy(out=vst[:], in_=ps[:])
                    po = mmp.tile([P, UC], f32)
                    for h in range(NH):
                        hs = slice(h * NBLK, (h + 1) * NBLK)
                        nc.tensor.matmul(
                            po[:, hs],
                            lhsT=vst[:],
                            rhs=uhi[:, hs],
                            start=True,
                            stop=True,
                        )
                    rows = P if j < fk else rk
                    osl = slice(j * UC, (j + 1) * UC)
                    eng = nc.vector if cp % 2 == 0 else nc.scalar
                    cp += 1
                    if eng is nc.vector:
                        eng.tensor_copy(out=ob[0:rows, osl], in_=po[0:rows, :])
                    else:
                        eng.copy(out=ob[0:rows, osl], in_=po[0:rows, :])
                if fk:
                    dst = out_dram[rowoff : rowoff + fk * P, :].rearrange(
                        "(a p) n -> p a n", p=P
                    )
                    src = ob[:, 0 : fk * UC].rearrange("p (a n) -> p a n", n=UC)
                    nc.sync.dma_start(out=dst, in_=src)
                if rk:
                    nc.sync.dma_start(
                        out=out_dram[rowoff + fk * P : rowoff + fk * P + rk, :],
                        in_=ob[0:rk, fk * UC : (fk + 1) * UC],
                    )
                tglob += tk
    nc.finalize()
    return nc


def _prep_items(ids):
    """Bucket item ids by static table ranges. Returns (mks, nks, perm,
    idx16 array [128, sum(mks)//16])."""
    b = ids // BWIDTH
    perm = np.argsort(b, kind="stable")
    sids = ids[perm]
    sb = b[perm]
    nks, chunks = [], []
    for k in range(NBUCKET):
        sel = sids[sb == k]
        n = len(sel)
        m = _ceil128(max(n, 1))
        loc = np.zeros(m, dtype=np.int16)
        loc[:n] = (sel - k * BWIDTH).astype(np.int16)
        nks.append(n)
        chunks.append(loc)
    mks = tuple(len(c) for c in chunks)
    idx16 = np.concatenate(chunks)
    wrapped = idx16.reshape(-1, 16).T            # [16, sum(mks)//16]
    rep = np.tile(wrapped, (8, 1))               # [128, ...]
    return mks, tuple(nks), perm, np.ascontiguousarray(rep)


def kernel(user_hiddens, item_hiddens, user_ids, item_ids, **_):
    user_hiddens = np.ascontiguousarray(user_hiddens, dtype=np.float32)
    item_hiddens = np.asarray(item_hiddens, dtype=np.float32)
    user_ids = np.asarray(user_ids)
    item_ids = np.asarray(item_ids)

    item_dup = np.empty((NUM_ITEMS, E), dtype=ml_dtypes.bfloat16)
    item_dup[:, 0:D] = item_hiddens
    item_dup[:, D:E] = item_dup[:, 0:D]

    preps = []
    for ci in range(RI):
        ids = item_ids[ci * IC : (ci + 1) * IC].astype(np.int64)
        preps.append(_prep_items(ids))
    # one program per distinct bucket-shape pair; RI=2 shapes must match to
    # stay SPMD, so build with the max per-bucket sizes padded identically
    mks = tuple(
        max(preps[ci][0][k] for ci in range(RI)) for k in range(NBUCKET)
    )
    if any(preps[ci][0] != mks for ci in range(RI)):
        # rebuild idx arrays padded to common shape
        new_preps = []
        for ci in range(RI):
            ids = item_ids[ci * IC : (ci + 1) * IC].astype(np.int64)
            b = ids // BWIDTH
            perm = np.argsort(b, kind="stable")
            sids = ids[perm]
            sb = b[perm]
            nks, chunks = [], []
            for k in range(NBUCKET):
                sel = sids[sb == k]
                n = len(sel)
                loc = np.zeros(mks[k], dtype=np.int16)
                loc[:n] = (sel - k * BWIDTH).astype(np.int16)
                nks.append(n)
                chunks.append(loc)
            idx16 = np.concatenate(chunks)
            wrapped = idx16.reshape(-1, 16).T
            rep = np.tile(wrapped, (8, 1))
            new_preps.append((mks, tuple(nks), perm, np.ascontiguousarray(rep)))
        preps = new_preps

    # nks may differ between the two item groups; the partial-row DMA
    # bookkeeping is per-core program state, so SPMD requires equal nks
    # too. Use per-ci programs only if they differ; normally the harness
    # inputs give one shape. Fall back: treat all rows as real (nks=mks)
    # and DMA pad rows into a padded out tensor? Simpler: require equal.
    nks0 = preps[0][1]
    same = all(preps[ci][1] == nks0 for ci in range(RI))
    if not same:
        # pad nks to mks: copy/DMA everything (incl. garbage pad rows) into
        # a padded output; host drops pads. Costs a few % extra copies/DMA.
        nks_use = mks
    else:
        nks_use = nks0

    ckey = (mks, nks_use)
    if ckey not in _cache:
        _cache.clear()
        _cache[ckey] = _build(mks, nks_use)
    nc = _cache[ckey]

    in_maps = []
    for c in range(N_CORES):
        cu, ci = divmod(c, RI)
        uc = user_ids[cu * UC : (cu + 1) * UC]
        uids_t = np.ascontiguousarray(uc.astype(np.int32).reshape(UT, P).T)
        in_maps.append(
            {
                "user_table": user_hiddens,
                "item_dup": item_dup,
                "uids": uids_t,
                "iidx": preps[ci][3],
            }
        )

    res = run_bass_kernel_spmd(nc, in_maps, list(range(N_CORES)))
    out = np.empty((BU, BI), dtype=np.float32)
    for c in range(N_CORES):
        cu, ci = divmod(c, RI)
        mks_ci, nks_ci, perm, _ = preps[ci]
        block = res.results[c]["out"]  # [IC, UC] fp16, bucket order rows
        if nks_use is mks:
            # padded mode: real rows are the first nks_ci[k] of each
            # bucket's padded region
            sel = []
            off = 0
            for k in range(NBUCKET):
                sel.extend(range(off, off + nks_ci[k]))
                off += mks[k]
            block = block[sel, :]
        out[
            cu * UC : (cu + 1) * UC, ci * IC + perm
        ] = block.T.astype(np.float32)
    return out


# revision 19
# speedup vs baseline: 1.0775x; 1.0775x over previous
"""Trainium2 Bass kernel for MF embedding-lookup + dot-product scoring.

out[u, i] = dot(user_hiddens[user_ids[u]], item_hiddens[item_ids[i]])

Sharding: 2D over 8 cores - 4 user groups (1024 users) x 2 item groups
(2048 items); tables replicated to every core's HBM. Per core:
  - users: 8 indirect-DMA gathers (128 f32 rows each) -> PE transpose ->
    bf16 cast -> uhi [64, 1024]
  - items: host converts the item table to bf16 with columns duplicated
    to 128 (256B rows, dma_gather's granularity) and range-buckets this
    core's item ids into 4 static 25600-row table slices; 4 bulk
    dma_gather calls (single_packet=False - large single packets crash
    the SWDGE) land 128-row bf16 tiles partition-major, one descriptor
    per row; int16 indices stay in range. ~1.2us/call vs ~1.1us per 128
    rows for indirect DMA.
  - per 128-item tile: bf16 PE transpose -> v^T, then 2 matmuls (K=64,
    N=512): lhsT = v^T, rhs = uhi halves -> f32 PSUM
  - PSUM -> SBUF fp16 casts alternate vector/scalar; per-bucket batched
    DMAs write only the real (non-pad) rows -> out [2048, 1024] fp16
Host un-permutes the bucketed item order, transposes, upcasts to f32,
and assembles the full [4096, 4096].
"""

import numpy as np
import ml_dtypes

import concourse.bacc as bacc
import concourse.bass as bass
import concourse.mybir as mybir
import concourse.tile as tile
from concourse.bass_utils import run_bass_kernel_spmd
from concourse.masks import make_identity

NUM_USERS = 1_000_000
NUM_ITEMS = 100_000
D = 64
E = 128             # bf16 item row width (256B, dma_gather granularity)
BU = 4096
BI = 4096
N_CORES = 8
RU = 4              # user groups
RI = 2              # item groups
UC = BU // RU       # users per core = 1024
IC = BI // RI       # items per core = 2048
P = 128
UT = UC // P        # user gather calls = 8
NBLK = 512          # matmul moving free dim (one PSUM bank of f32)
NH = UC // NBLK     # user halves per item tile = 2
NBUCKET = 4
BWIDTH = 25_600     # static item-table range per bucket (< 32768)

# Spread SWDGE DMAs across the 4 descriptor-ring queues (queue = DMASW
# sem lane % 4, so each of the 8 lanes only ever sees one queue). CoreSim
# models indirect DMA as queue-0 regardless of the encoded ring, so the
# sim check flips this off; data semantics are queue-independent.
MULTIQ = True

_cache = {}


def _ceil128(n):
    return (n + P - 1) // P * P


def _build(mks, nks):
    """mks: per-bucket padded index counts (x128); nks: real counts."""
    nc = bacc.Bacc(num_swdge_queues=4)
    ut_dram = nc.dram_tensor(
        "user_table", [NUM_USERS, D], mybir.dt.float32, kind="ExternalInput"
    )
    it_dram = nc.dram_tensor(
        "item_dup", [NUM_ITEMS, E], mybir.dt.bfloat16, kind="ExternalInput"
    )
    uid_dram = nc.dram_tensor("uids", [P, UT], mybir.dt.int32, kind="ExternalInput")
    icols = sum(mks) // 16
    iidx_dram = nc.dram_tensor(
        "iidx", [P, icols], mybir.dt.int16, kind="ExternalInput"
    )
    out_rows = sum(nks)
    out_dram = nc.dram_tensor(
        "out", [out_rows, UC], mybir.dt.float16, kind="ExternalOutput"
    )

    f32 = mybir.dt.float32
    bf16 = mybir.dt.bfloat16
    fp16 = mybir.dt.float16

    with tile.TileContext(nc) as tc:
        with (
            tc.tile_pool(name="const", bufs=1) as constp,
            tc.tile_pool(name="idx", bufs=1) as idxp,
            tc.tile_pool(name="gath", bufs=1) as gathp,
            tc.tile_pool(name="ops", bufs=1) as opsp,
            tc.tile_pool(name="vt", bufs=4) as vtp,
            tc.tile_pool(name="tp", bufs=2, space="PSUM") as tpp,
            tc.tile_pool(name="mm", bufs=3, space="PSUM") as mmp,
            tc.tile_pool(name="outp", bufs=2) as outp,
        ):
            ident = constp.tile([P, P], f32)
            make_identity(nc, ident[:])
            ident_bf = constp.tile([P, P], bf16)
            make_identity(nc, ident_bf[:])

            uids = idxp.tile([P, UT], mybir.dt.int32)
            iidx = idxp.tile([P, icols], mybir.dt.int16)
            nc.sync.dma_start(out=uids[:], in_=uid_dram[:])
            nc.sync.dma_start(out=iidx[:], in_=iidx_dram[:])

            gu = gathp.tile([P, UT * D], f32)
            ntiles = sum(mks) // P
            vmov = gathp.tile([P, ntiles * E], bf16)

            def user_gather(t):
                inst = nc.gpsimd.indirect_dma_start(
                    out=gu[:, t * D : (t + 1) * D],
                    out_offset=None,
                    in_=ut_dram[:],
                    in_offset=bass.IndirectOffsetOnAxis(
                        ap=uids[:, t : t + 1], axis=0
                    ),
                )
                q = (t % 8) % 4
                if MULTIQ and q:
                    inst.ins.queue = f"qPoolDynamic{q}"

            def item_gather(k):
                # one 128-row call per tile, rotating SWDGE queues 1-3 so
                # each call fits the descriptor ring and the rings drain
                # in parallel while GpSimd keeps issuing
                toff = sum(mks[:k]) // P
                for j in range(mks[k] // P):
                    t = toff + j
                    g = UT + t  # global Pool-DMA index (users first)
                    nc.gpsimd.dma_gather(
                        out_ap=vmov[:, t * E : (t + 1) * E].rearrange(
                            "p (o n) -> p o n", n=E
                        ),
                        in_ap=it_dram[
                            k * BWIDTH : min((k + 1) * BWIDTH, NUM_ITEMS), :
                        ],
                        idxs_ap=iidx[:, t * 8 : (t + 1) * 8],
                        num_idxs=P,
                        num_idxs_reg=P,
                        elem_size=E,
                        transpose=False,
                        single_packet=False,
                        queue_num=(g % 8) % 4 if MULTIQ else 0,
                    )

            # gpsimd issue order: the serial user stream first (its data is
            # needed by every matmul), then one ucode-library switch, then
            # the multi-queue item stream whose rings drain in parallel
            for t in range(UT):
                user_gather(t)
            for k in range(NBUCKET):
                item_gather(k)

            # --- user prologue: transpose + bf16 cast -> uhi [64, 1024] ---
            uhi = opsp.tile([D, UC], bf16)
            for t in range(UT):
                ps = tpp.tile([D, P], f32)
                nc.tensor.transpose(ps[:], gu[:, t * D : (t + 1) * D], ident[:])
                nc.vector.tensor_copy(
                    out=uhi[:, t * P : (t + 1) * P], in_=ps[:]
                )

            # --- item stream: bf16 transpose -> matmuls ---
            cp = 0  # copy-engine rotation counter
            tglob = 0
            for k in range(NBUCKET):
                tk = mks[k] // P          # tiles in this bucket
                fk = nks[k] // P          # full tiles
                rk = nks[k] % P           # real rows in partial tile
                rowoff = sum(nks[:k])     # output row offset
                ob = outp.tile([P, tk * UC], fp16)
                for j in range(tk):
                    t = tglob + j
                    ps = tpp.tile([D, P], bf16)
                    nc.tensor.transpose(
                        ps[:], vmov[:, t * E : t * E + D], ident_bf[:]
                    )
                    vst = vtp.tile([D, P], bf16)
                    nc.scalar.copy(out=vst[:], in_=ps[:])
                    po = mmp.tile([P, UC], f32)
                    for h in range(NH):
                        hs = slice(h * NBLK, (h + 1) * NBLK)
                        nc.tensor.matmul(
                            po[:, hs],
                            lhsT=vst[:],
                            rhs=uhi[:, hs],
                            start=True,
                            stop=True,
                        )
                    rows = P if j < fk else rk
                    osl = slice(j * UC, (j + 1) * UC)
                    eng = nc.vector if cp % 2 == 0 else nc.scalar
                    cp += 1
                    if eng is nc.vector:
                        eng.tensor_copy(out=ob[0:rows, osl], in_=po[0:rows, :])
                    else:
                        eng.copy(out=ob[0:rows, osl], in_=po[0:rows, :])
                if fk:
                    dst = out_dram[rowoff : rowoff + fk * P, :].rearrange(
                        "(a p) n -> p a n", p=P
                    )
                    src = ob[:, 0 : fk * UC].rearrange("p (a n) -> p a n", n=UC)
                    nc.sync.dma_start(out=dst, in_=src)
                if rk:
                    nc.sync.dma_start(
                        out=out_dram[rowoff + fk * P : rowoff + fk * P + rk, :],
                        in_=ob[0:rk, fk * UC : (fk + 1) * UC],
                    )
                tglob += tk
    nc.finalize()
    return nc


def _prep_items(ids):
    """Bucket item ids by static table ranges. Returns (mks, nks, perm,
    idx16 array [128, sum(mks)//16])."""
    b = ids // BWIDTH
    perm = np.argsort(b, kind="stable")
    sids = ids[perm]
    sb = b[perm]
    nks, chunks = [], []
    for k in range(NBUCKET):
        sel = sids[sb == k]
        n = len(sel)
        m = _ceil128(max(n, 1))
        loc = np.zeros(m, dtype=np.int16)
        loc[:n] = (sel - k * BWIDTH).astype(np.int16)
        nks.append(n)
        chunks.append(loc)
    mks = tuple(len(c) for c in chunks)
    idx16 = np.concatenate(chunks)
    wrapped = idx16.reshape(-1, 16).T            # [16, sum(mks)//16]
    rep = np.tile(wrapped, (8, 1))               # [128, ...]
    return mks, tuple(nks), perm, np.ascontiguousarray(rep)


def kernel(user_hiddens, item_hiddens, user_ids, item_ids, **_):
    user_hiddens = np.ascontiguousarray(user_hiddens, dtype=np.float32)
    item_hiddens = np.asarray(item_hiddens, dtype=np.float32)
    user_ids = np.asarray(user_ids)
    item_ids = np.asarray(item_ids)

    item_dup = np.empty((NUM_ITEMS, E), dtype=ml_dtypes.bfloat16)
    item_dup[:, 0:D] = item_hiddens
    item_dup[:, D:E] = item_dup[:, 0:D]

    preps = []
    for ci in range(RI):
        ids = item_ids[ci * IC : (ci + 1) * IC].astype(np.int64)
        preps.append(_prep_items(ids))
    # one program per distinct bucket-shape pair; RI=2 shapes must match to
    # stay SPMD, so build with the max per-bucket sizes padded identically
    mks = tuple(
        max(preps[ci][0][k] for ci in range(RI)) for k in range(NBUCKET)
    )
    if any(preps[ci][0] != mks for ci in range(RI)):
        # rebuild idx arrays padded to common shape
        new_preps = []
        for ci in range(RI):
            ids = item_ids[ci * IC : (ci + 1) * IC].astype(np.int64)
            b = ids // BWIDTH
            perm = np.argsort(b, kind="stable")
            sids = ids[perm]
            sb = b[perm]
            nks, chunks = [], []
            for k in range(NBUCKET):
                sel = sids[sb == k]
                n = len(sel)
                loc = np.zeros(mks[k], dtype=np.int16)
                loc[:n] = (sel - k * BWIDTH).astype(np.int16)
                nks.append(n)
                chunks.append(loc)
            idx16 = np.concatenate(chunks)
            wrapped = idx16.reshape(-1, 16).T
            rep = np.tile(wrapped, (8, 1))
            new_preps.append((mks, tuple(nks), perm, np.ascontiguousarray(rep)))
        preps = new_preps

    # nks may differ between the two item groups; the partial-row DMA
    # bookkeeping is per-core program state, so SPMD requires equal nks
    # too. Use per-ci programs only if they differ; normally the harness
    # inputs give one shape. Fall back: treat all rows as real (nks=mks)
    # and DMA pad rows into a padded out tensor? Simpler: require equal.
    nks0 = preps[0][1]
    same = all(preps[ci][1] == nks0 for ci in range(RI))
    if not same:
        # pad nks to mks: copy/DMA everything (incl. garbage pad rows) into
        # a padded output; host drops pads. Costs a few % extra copies/DMA.
        nks_use = mks
    else:
        nks_use = nks0

    ckey = (mks, nks_use)
    if ckey not in _cache:
        _cache.clear()
        _cache[ckey] = _build(mks, nks_use)
    nc = _cache[ckey]

    in_maps = []
    for c in range(N_CORES):
        cu, ci = divmod(c, RI)
        uc = user_ids[cu * UC : (cu + 1) * UC]
        uids_t = np.ascontiguousarray(uc.astype(np.int32).reshape(UT, P).T)
        in_maps.append(
            {
                "user_table": user_hiddens,
                "item_dup": item_dup,
                "uids": uids_t,
                "iidx": preps[ci][3],
            }
        )

    res = run_bass_kernel_spmd(nc, in_maps, list(range(N_CORES)))
    out = np.empty((BU, BI), dtype=np.float32)
    for c in range(N_CORES):
        cu, ci = divmod(c, RI)
        mks_ci, nks_ci, perm, _ = preps[ci]
        block = res.results[c]["out"]  # [IC, UC] fp16, bucket order rows
        if nks_use is mks:
            # padded mode: real rows are the first nks_ci[k] of each
            # bucket's padded region
            sel = []
            off = 0
            for k in range(NBUCKET):
                sel.extend(range(off, off + nks_ci[k]))
                off += mks[k]
            block = block[sel, :]
        out[
            cu * UC : (cu + 1) * UC, ci * IC + perm
        ] = block.T.astype(np.float32)
    return out


# revision 27
# speedup vs baseline: 1.2287x; 1.1403x over previous
"""Trainium2 Bass kernel for MF embedding-lookup + dot-product scoring.

out[u, i] = dot(user_hiddens[user_ids[u]], item_hiddens[item_ids[i]])

Sharding: 2D over 8 cores - 4 user groups (1024 users) x 2 item groups
(2048 items); tables replicated to every core's HBM. Per core:
  - users: 8 indirect-DMA gathers (128 f32 rows each) -> PE transpose ->
    bf16 cast -> uhi [64, 1024]
  - items: host converts the item table to bf16 with columns duplicated
    to 128 (256B rows, dma_gather's granularity) and range-buckets this
    core's item ids into 4 static 25600-row table slices; 4 bulk
    dma_gather calls (single_packet=False - large single packets crash
    the SWDGE) land 128-row bf16 tiles partition-major, one descriptor
    per row; int16 indices stay in range. ~1.2us/call vs ~1.1us per 128
    rows for indirect DMA.
  - per 128-item tile: bf16 PE transpose -> v^T, then 2 matmuls (K=64,
    N=512): lhsT = v^T, rhs = uhi halves -> f32 PSUM
  - PSUM -> SBUF fp16 casts alternate vector/scalar; per-bucket batched
    DMAs write only the real (non-pad) rows -> out [2048, 1024] fp16
Host un-permutes the bucketed item order, transposes, upcasts to f32,
and assembles the full [4096, 4096].
"""

import numpy as np
import ml_dtypes

import concourse.bacc as bacc
import concourse.bass as bass
import concourse.library_config as library_config
import concourse.mybir as mybir
import concourse.tile as tile
from concourse.bass_utils import run_bass_kernel_spmd
from concourse.masks import make_identity

NUM_USERS = 1_000_000
NUM_ITEMS = 100_000
D = 64
E = 128             # bf16 item row width (256B, dma_gather granularity)
BU = 4096
BI = 4096
N_CORES = 8
RU = 4              # user groups
RI = 2              # item groups
UC = BU // RU       # users per core = 1024
IC = BI // RI       # items per core = 2048
P = 128
UT = UC // P        # user gather calls = 8
NBLK = 512          # matmul moving free dim (one PSUM bank of f32)
NH = UC // NBLK     # user halves per item tile = 2
NBUCKET = 4
BWIDTH = 25_600     # static item-table range per bucket (< 32768)

# Spread SWDGE DMAs across the 4 descriptor-ring queues (queue = DMASW
# sem lane % 4, so each of the 8 lanes only ever sees one queue). CoreSim
# models indirect DMA as queue-0 regardless of the encoded ring, so the
# sim check flips this off; data semantics are queue-independent.
MULTIQ = True
WARMUP = False
PRELOAD_LIB = True

_cache = {}


def _ceil128(n):
    return (n + P - 1) // P * P


def _build(mks, nks):
    """mks: per-bucket padded index counts (x128); nks: real counts."""
    nc = bacc.Bacc(num_swdge_queues=4)
    ut_dram = nc.dram_tensor(
        "user_table", [NUM_USERS, D], mybir.dt.float32, kind="ExternalInput"
    )
    it_dram = nc.dram_tensor(
        "item_dup", [NUM_ITEMS, E], mybir.dt.bfloat16, kind="ExternalInput"
    )
    uid_dram = nc.dram_tensor("uids", [P, UT], mybir.dt.int32, kind="ExternalInput")
    icols = sum(mks) // 16
    iidx_dram = nc.dram_tensor(
        "iidx", [P, icols], mybir.dt.int16, kind="ExternalInput"
    )
    out_rows = sum(nks)
    out_dram = nc.dram_tensor(
        "out", [out_rows, UC], mybir.dt.float16, kind="ExternalOutput"
    )

    f32 = mybir.dt.float32
    bf16 = mybir.dt.bfloat16
    fp16 = mybir.dt.float16

    with tile.TileContext(nc) as tc:
        with (
            tc.tile_pool(name="const", bufs=1) as constp,
            tc.tile_pool(name="idx", bufs=1) as idxp,
            tc.tile_pool(name="gath", bufs=1) as gathp,
            tc.tile_pool(name="ops", bufs=1) as opsp,
            tc.tile_pool(name="vt", bufs=4) as vtp,
            tc.tile_pool(name="tp", bufs=2, space="PSUM") as tpp,
            tc.tile_pool(name="mm", bufs=3, space="PSUM") as mmp,
            tc.tile_pool(name="outp", bufs=2) as outp,
        ):
            # preload the gather ucode library while the index DMA and the
            # indirect user gathers (resident ucode) run - the load takes
            # ~7us and otherwise serializes between gather phases
            if PRELOAD_LIB:
                nc.gpsimd.load_library(library_config.mlp)

            ident = constp.tile([P, P], f32)
            make_identity(nc, ident[:])
            ident_bf = constp.tile([P, P], bf16)
            make_identity(nc, ident_bf[:])

            uids = idxp.tile([P, UT], mybir.dt.int32)
            iidx = idxp.tile([P, icols], mybir.dt.int16)
            nc.sync.dma_start(out=uids[:], in_=uid_dram[:])
            nc.sync.dma_start(out=iidx[:], in_=iidx_dram[:])

            gu = gathp.tile([P, UT * D], f32)
            ntiles = sum(mks) // P
            vmov = gathp.tile([P, ntiles * E], bf16)

            def user_gather(t):
                inst = nc.gpsimd.indirect_dma_start(
                    out=gu[:, t * D : (t + 1) * D],
                    out_offset=None,
                    in_=ut_dram[:],
                    in_offset=bass.IndirectOffsetOnAxis(
                        ap=uids[:, t : t + 1], axis=0
                    ),
                )
                q = (t % 8) % 4
                if MULTIQ and q:
                    inst.ins.queue = f"qPoolDynamic{q}"

            def item_gather(k):
                # one 128-row call per tile, rotating SWDGE queues 1-3 so
                # each call fits the descriptor ring and the rings drain
                # in parallel while GpSimd keeps issuing
                toff = sum(mks[:k]) // P
                for j in range(mks[k] // P):
                    t = toff + j
                    g = UT + t  # global Pool-DMA index (users first)
                    nc.gpsimd.dma_gather(
                        out_ap=vmov[:, t * E : (t + 1) * E].rearrange(
                            "p (o n) -> p o n", n=E
                        ),
                        in_ap=it_dram[
                            k * BWIDTH : min((k + 1) * BWIDTH, NUM_ITEMS), :
                        ],
                        idxs_ap=iidx[:, t * 8 : (t + 1) * 8],
                        num_idxs=P,
                        num_idxs_reg=P,
                        elem_size=E,
                        transpose=False,
                        single_packet=False,
                        queue_num=(g % 8) % 4 if MULTIQ else 0,
                    )

            # gpsimd issue order: the serial user stream first (its data is
            # needed by every matmul), then the multi-queue item stream
            # whose rings drain in parallel
            for t in range(UT):
                user_gather(t)
            for k in range(NBUCKET):
                item_gather(k)

            # HAM warm-up: ~3.7us of back-to-back PE activity releases the
            # clock throttle (1.2 -> 2.4 GHz); the per-gather transposes and
            # the matmul stream then sustain it
            if WARMUP:
                for _ in range(34):
                    nc.tensor.ldweights(ident_bf[:])

            # --- user prologue: transpose + bf16 cast -> uhi [64, 1024] ---
            uhi = opsp.tile([D, UC], bf16)
            for t in range(UT):
                ps = tpp.tile([D, P], f32)
                nc.tensor.transpose(ps[:], gu[:, t * D : (t + 1) * D], ident[:])
                nc.vector.tensor_copy(
                    out=uhi[:, t * P : (t + 1) * P], in_=ps[:]
                )

            # --- item stream: bf16 transpose -> matmuls ---
            cp = 0  # copy-engine rotation counter
            tglob = 0
            for k in range(NBUCKET):
                tk = mks[k] // P          # tiles in this bucket
                fk = nks[k] // P          # full tiles
                rk = nks[k] % P           # real rows in partial tile
                rowoff = sum(nks[:k])     # output row offset
                ob = outp.tile([P, tk * UC], fp16)
                for j in range(tk):
                    t = tglob + j
                    ps = tpp.tile([D, P], bf16)
                    nc.tensor.transpose(
                        ps[:], vmov[:, t * E : t * E + D], ident_bf[:]
                    )
                    vst = vtp.tile([D, P], bf16)
                    if t % 2 == 0:
                        nc.vector.tensor_copy(out=vst[:], in_=ps[:])
                    else:
                        nc.scalar.copy(out=vst[:], in_=ps[:])
                    po = mmp.tile([P, UC], f32)
                    for h in range(NH):
                        hs = slice(h * NBLK, (h + 1) * NBLK)
                        nc.tensor.matmul(
                            po[:, hs],
                            lhsT=vst[:],
                            rhs=uhi[:, hs],
                            start=True,
                            stop=True,
                        )
                    # split the PSUM->SBUF fp16 cast across both engines so
                    # PSUM banks recycle faster and the copy wall is shared
                    rows = P if j < fk else rk
                    osl0 = slice(j * UC, j * UC + NBLK)
                    osl1 = slice(j * UC + NBLK, (j + 1) * UC)
                    nc.scalar.copy(out=ob[0:rows, osl0], in_=po[0:rows, 0:NBLK])
                    nc.vector.tensor_copy(
                        out=ob[0:rows, osl1], in_=po[0:rows, NBLK:UC]
                    )
                if fk:
                    dst = out_dram[rowoff : rowoff + fk * P, :].rearrange(
                        "(a p) n -> p a n", p=P
                    )
                    src = ob[:, 0 : fk * UC].rearrange("p (a n) -> p a n", n=UC)
                    nc.sync.dma_start(out=dst, in_=src)
                if rk:
                    nc.sync.dma_start(
                        out=out_dram[rowoff + fk * P : rowoff + fk * P + rk, :],
                        in_=ob[0:rk, fk * UC : (fk + 1) * UC],
                    )
                tglob += tk
    nc.finalize()
    return nc


def _prep_items(ids):
    """Bucket item ids by static table ranges. Returns (mks, nks, perm,
    idx16 array [128, sum(mks)//16])."""
    b = ids // BWIDTH
    perm = np.argsort(b, kind="stable")
    sids = ids[perm]
    sb = b[perm]
    nks, chunks = [], []
    for k in range(NBUCKET):
        sel = sids[sb == k]
        n = len(sel)
        m = _ceil128(max(n, 1))
        loc = np.zeros(m, dtype=np.int16)
        loc[:n] = (sel - k * BWIDTH).astype(np.int16)
        nks.append(n)
        chunks.append(loc)
    mks = tuple(len(c) for c in chunks)
    idx16 = np.concatenate(chunks)
    wrapped = idx16.reshape(-1, 16).T            # [16, sum(mks)//16]
    rep = np.tile(wrapped, (8, 1))               # [128, ...]
    return mks, tuple(nks), perm, np.ascontiguousarray(rep)


def kernel(user_hiddens, item_hiddens, user_ids, item_ids, **_):
    user_hiddens = np.ascontiguousarray(user_hiddens, dtype=np.float32)
    item_hiddens = np.asarray(item_hiddens, dtype=np.float32)
    user_ids = np.asarray(user_ids)
    item_ids = np.asarray(item_ids)

    item_dup = np.empty((NUM_ITEMS, E), dtype=ml_dtypes.bfloat16)
    item_dup[:, 0:D] = item_hiddens
    item_dup[:, D:E] = item_dup[:, 0:D]

    preps = []
    for ci in range(RI):
        ids = item_ids[ci * IC : (ci + 1) * IC].astype(np.int64)
        preps.append(_prep_items(ids))
    # one program per distinct bucket-shape pair; RI=2 shapes must match to
    # stay SPMD, so build with the max per-bucket sizes padded identically
    mks = tuple(
        max(preps[ci][0][k] for ci in range(RI)) for k in range(NBUCKET)
    )
    if any(preps[ci][0] != mks for ci in range(RI)):
        # rebuild idx arrays padded to common shape
        new_preps = []
        for ci in range(RI):
            ids = item_ids[ci * IC : (ci + 1) * IC].astype(np.int64)
            b = ids // BWIDTH
            perm = np.argsort(b, kind="stable")
            sids = ids[perm]
            sb = b[perm]
            nks, chunks = [], []
            for k in range(NBUCKET):
                sel = sids[sb == k]
                n = len(sel)
                loc = np.zeros(mks[k], dtype=np.int16)
                loc[:n] = (sel - k * BWIDTH).astype(np.int16)
                nks.append(n)
                chunks.append(loc)
            idx16 = np.concatenate(chunks)
            wrapped = idx16.reshape(-1, 16).T
            rep = np.tile(wrapped, (8, 1))
            new_preps.append((mks, tuple(nks), perm, np.ascontiguousarray(rep)))
        preps = new_preps

    # nks may differ between the two item groups; the partial-row DMA
    # bookkeeping is per-core program state, so SPMD requires equal nks
    # too. Use per-ci programs only if they differ; normally the harness
    # inputs give one shape. Fall back: treat all rows as real (nks=mks)
    # and DMA pad rows into a padded out tensor? Simpler: require equal.
    nks0 = preps[0][1]
    same = all(preps[ci][1] == nks0 for ci in range(RI))
    if not same:
        # pad nks to mks: copy/DMA everything (incl. garbage pad rows) into
        # a padded output; host drops pads. Costs a few % extra copies/DMA.
        nks_use = mks
    else:
        nks_use = nks0

    ckey = (mks, nks_use)
    if ckey not in _cache:
        _cache.clear()
        _cache[ckey] = _build(mks, nks_use)
    nc = _cache[ckey]

    in_maps = []
    for c in range(N_CORES):
        cu, ci = divmod(c, RI)
        uc = user_ids[cu * UC : (cu + 1) * UC]
        uids_t = np.ascontiguousarray(uc.astype(np.int32).reshape(UT, P).T)
        in_maps.append(
            {
                "user_table": user_hiddens,
                "item_dup": item_dup,
                "uids": uids_t,
                "iidx": preps[ci][3],
            }
        )

    res = run_bass_kernel_spmd(nc, in_maps, list(range(N_CORES)))
    out = np.empty((BU, BI), dtype=np.float32)
    for c in range(N_CORES):
        cu, ci = divmod(c, RI)
        mks_ci, nks_ci, perm, _ = preps[ci]
        block = res.results[c]["out"]  # [IC, UC] fp16, bucket order rows
        if nks_use is mks:
            # padded mode: real rows are the first nks_ci[k] of each
            # bucket's padded region
            sel = []
            off = 0
            for k in range(NBUCKET):
                sel.extend(range(off, off + nks_ci[k]))
                off += mks[k]
            block = block[sel, :]
        out[
            cu * UC : (cu + 1) * UC, ci * IC + perm
        ] = block.T.astype(np.float32)
    return out
